# revision 1
# baseline (speedup 1.0000x reference)
"""Trainium2 Bass kernel for a single-head transformer encoder layer.

Problem shapes (hardcoded): B=4, S=4096, D=512, D_FFN=2048, fp32.
Sharding: 8 cores; core c handles batch b=c//2, query-row half h=c%2
(2048 q rows each). K/V for the batch's full sequence (4096 rows) are
projected on-core (duplicated across the 2 cores sharing a batch).

v2 structure (all matmuls float32r: 1 cycle/row, ~1.5e-4 rounding):
  pass 1: load q/k/v, PE-transpose to feature-major, project.
          QT [P,DC,M], KT [P,DC,S], V [P,S/P,D] stay resident in SBUF.
  pass 2: per 512-q block: scores S^T = lhsT(KT)@rhs(QT) -> exp on ACT
          (no max subtraction; scores ~ N(0,1)) -> P^T chunks feed the
          attn matmul (lhsT=PT, rhs=V) accumulating 32 chunks in PSUM;
          row sums ride along via a ones matmul. One drain per q block
          fused with 1/rsum, then +x, LN1; h rows spilled to DRAM.
  pass 3: FFN per 512-row block: re-read h, transpose, FFN1 (relu+bias
          fused in ACT copyback), FFN2, +b2 +h residual, LN2, store.
"""

import math
import threading
from contextlib import ExitStack

import numpy as np

import concourse.bass as bass
import concourse.tile as tile
from concourse import bacc, mybir
from concourse.bass_utils import run_bass_kernel_spmd
from concourse.masks import make_identity

P = 128
B, S, D = 4, 4096, 512
F = 4 * D                    # 2048
M = S // 2                   # q rows per core
DC = D // P                  # 4 feature chunks
FC = F // P                  # 16 ffn chunks
KB = 512                     # load-block rows
QB = 512                     # q-block cols
NQB = M // QB                # 4
SC = S // P                  # 32 k chunks
RC = M // P                  # 16 row chunks per core
EPS = 1e-5
SCALE = 1.0 / math.sqrt(D)
f32 = mybir.dt.float32
f32r = mybir.dt.float32r
bf16 = mybir.dt.bfloat16
N_CORES = 8


def _ln_stats(nc, pool, t):
    fmax = nc.vector.BN_STATS_FMAX
    if D <= fmax:
        stats = pool.tile([P, nc.vector.BN_STATS_DIM], f32, tag="ln_stats")
        nc.vector.bn_stats(out=stats[:], in_=t[:])
        mv = pool.tile([P, nc.vector.BN_AGGR_DIM], f32, tag="ln_mv")
        nc.vector.bn_aggr(out=mv[:], in_=stats[:])
    else:
        sub = math.gcd(fmax, D)
        nsub = D // sub
        tr = t.rearrange("p (n s) -> p n s", s=sub)
        stats = pool.tile([P, nsub, nc.vector.BN_STATS_DIM], f32, tag="ln_stats")
        for i in range(nsub):
            nc.vector.bn_stats(out=stats[:, i, :], in_=tr[:, i, :])
        mv = pool.tile([P, nc.vector.BN_AGGR_DIM], f32, tag="ln_mv")
        nc.vector.bn_aggr(out=mv[:], in_=stats[:])
    return mv[:, 0:1], mv[:, 1:2]


def _apply_ln(nc, pool, t, eps_t, gamma_bc, beta_bc):
    mean, var = _ln_stats(nc, pool, t)
    nc.scalar.activation(out=var, in_=var,
                         func=mybir.ActivationFunctionType.Sqrt,
                         bias=eps_t[:], scale=1.0, alpha=0.0)
    nc.vector.reciprocal(out=var, in_=var)
    nc.vector.tensor_scalar(out=t[:], in0=t[:], scalar1=mean, scalar2=var,
                            op0=mybir.AluOpType.subtract,
                            op1=mybir.AluOpType.mult)
    nc.vector.tensor_mul(out=t[:], in0=t[:], in1=gamma_bc[:])
    nc.vector.tensor_add(out=t[:], in0=t[:], in1=beta_bc[:])


def _bcast_load(nc, pool, vec_ap, n, tag):
    t = pool.tile([P, n], f32, tag=tag)
    src = bass.AP(tensor=vec_ap.tensor, offset=vec_ap.offset,
                  ap=[[0, P]] + list(vec_ap.ap))
    nc.gpsimd.dma_start(out=t[:], in_=src)
    return t


def _fm_load(nc, pool, vec_ap, chunks, tag):
    t = pool.tile([P, chunks], f32, tag=tag)
    nc.sync.dma_start(t[:], vec_ap.rearrange("(c p) -> p c", p=P))
    return t


def _load_w_fm(nc, pool, raw_pool, w_ap, kchunks, nout, tag):
    t = pool.tile([P, kchunks, nout], f32r, tag=tag)
    wr = w_ap.rearrange("(c p) n -> p c n", p=P)
    for c in range(kchunks):
        raw = raw_pool.tile([P, nout], f32, tag="w_raw")
        nc.sync.dma_start(raw[:], wr[:, c, :])
        nc.vector.tensor_copy(t[:, c, :], raw[:])
    return t


def _transpose_rows(nc, ps_pool, ident, nat, fm, rt):
    """PE-transpose nat [P,512] into fm[:, :, rt*P:(rt+1)*P] via one
    4-quadrant PSUM bank and a single batched DVE drain."""
    pst = ps_pool.tile([P, DC, P], f32, tag="ps_tp")
    for dc in range(DC):
        nc.tensor.transpose(pst[:, dc, :], nat[:, dc * P:(dc + 1) * P], ident)
    nc.vector.tensor_copy(fm[:, :, rt * P:(rt + 1) * P], pst[:])


def build_program():
    nc = bacc.Bacc()
    q = nc.dram_tensor("q", [M, D], f32, kind="ExternalInput")
    k = nc.dram_tensor("k", [S, D], f32, kind="ExternalInput")
    v = nc.dram_tensor("v", [S, D], f32, kind="ExternalInput")
    x = nc.dram_tensor("x", [M, D], f32, kind="ExternalInput")
    Wq = nc.dram_tensor("Wq", [D, D], f32, kind="ExternalInput")
    Wk = nc.dram_tensor("Wk", [D, D], f32, kind="ExternalInput")
    Wv = nc.dram_tensor("Wv", [D, D], f32, kind="ExternalInput")
    bq = nc.dram_tensor("bq", [D], f32, kind="ExternalInput")
    bk = nc.dram_tensor("bk", [D], f32, kind="ExternalInput")
    bv = nc.dram_tensor("bv", [D], f32, kind="ExternalInput")
    g1 = nc.dram_tensor("gamma1", [D], f32, kind="ExternalInput")
    be1 = nc.dram_tensor("beta1", [D], f32, kind="ExternalInput")
    W1 = nc.dram_tensor("W1", [D, F], f32, kind="ExternalInput")
    b1 = nc.dram_tensor("b1", [F], f32, kind="ExternalInput")
    W2 = nc.dram_tensor("W2", [F, D], f32, kind="ExternalInput")
    b2 = nc.dram_tensor("b2", [D], f32, kind="ExternalInput")
    g2 = nc.dram_tensor("gamma2", [D], f32, kind="ExternalInput")
    be2 = nc.dram_tensor("beta2", [D], f32, kind="ExternalInput")
    out = nc.dram_tensor("out", [M, D], f32, kind="ExternalOutput")

    with tile.TileContext(nc) as tc, ExitStack() as ctx:
        g_pool = ctx.enter_context(tc.tile_pool(name="glob", bufs=1))
        io = ctx.enter_context(tc.tile_pool(name="io", bufs=3))
        htp = ctx.enter_context(tc.tile_pool(name="htp", bufs=2))
        wp = ctx.enter_context(tc.tile_pool(name="wp", bufs=1))
        ep = ctx.enter_context(tc.tile_pool(name="ep", bufs=2))
        ps_a = ctx.enter_context(tc.tile_pool(name="ps_a", bufs=2, space="PSUM"))

        ident_t = g_pool.tile([P, P], f32, tag="ident")
        make_identity(nc, ident_t[:])
        ident = ident_t[:]
        ones32 = g_pool.tile([P, 4], f32, tag="ones32")
        nc.vector.memset(ones32[:], 1.0)
        ones_r = g_pool.tile([P, 4], bf16, tag="ones")
        nc.vector.tensor_copy(ones_r[:], ones32[:])
        eps_t = g_pool.tile([P, 1], f32, tag="eps")
        nc.vector.memset(eps_t[:], EPS)
        h_full = g_pool.tile([P, RC, D], f32, tag="h_full")

        with ExitStack() as actx:
            attn = actx.enter_context(tc.tile_pool(name="attn", bufs=1))
            qt_full = attn.tile([P, DC, M], bf16, tag="qt_full")
            kt_full = attn.tile([P, DC, S], bf16, tag="kt_full")
            v_full = attn.tile([P, SC, D], bf16, tag="v_full")

            # ---------- pass 1: load + transpose + project ----------
            with ExitStack() as p1ctx:
                p1 = p1ctx.enter_context(tc.tile_pool(name="ph1", bufs=1))
                tp = p1ctx.enter_context(tc.tile_pool(name="tp", bufs=2))
                tpn = p1ctx.enter_context(tc.tile_pool(name="tpn", bufs=2))
                ps_t1 = p1ctx.enter_context(
                    tc.tile_pool(name="ps_t1", bufs=3, space="PSUM"))
                ps_p = p1ctx.enter_context(
                    tc.tile_pool(name="ps_p", bufs=3, space="PSUM"))

                bq_fm = _fm_load(nc, p1, bq[:], DC, "bq")
                bk_fm = _fm_load(nc, p1, bk[:], DC, "bk")
                bv_bc = _bcast_load(nc, attn, bv[:], D, "bv")

                # q and k: feature-major projections (lhsT = W chunk)
                for name, src, rows, w_ap, b_fm, dst in (
                        ("k", k, S, Wk, None, kt_full),
                        ("q", q, M, Wq, None, qt_full)):
                    w_sb = p1.tile([P, DC, D], f32r, tag="w_sb", name="w_sb")
                    nc.gpsimd.dma_start(
                        w_sb[:], w_ap.rearrange("(c p) n -> p c n", p=P))
                    b_fm = bq_fm if name == "q" else bk_fm
                    for j in range(rows // KB):
                        fmr = tp.tile([P, DC, KB], f32r, tag="in_fm",
                                      name=name + "_fm")
                        for half in range(2):
                            nat2 = tpn.tile([P, 2, D], f32, tag="in_nat4",
                                            name=name + "_nat")
                            base = j * KB + half * 2 * P
                            nc.sync.dma_start(
                                nat2[:], src[base:base + 2 * P, :].rearrange(
                                    "(t p) d -> p t d", p=P))
                            for rt in range(2):
                                _transpose_rows(nc, ps_t1, ident,
                                                nat2[:, rt, :], fmr,
                                                half * 2 + rt)
                        for m in range(DC):
                            psp = ps_p.tile([P, KB], f32, tag="ps_proj")
                            for kc in range(DC):
                                nc.tensor.matmul(
                                    psp[:],
                                    lhsT=w_sb[:, kc, m * P:(m + 1) * P],
                                    rhs=fmr[:, kc, :],
                                    start=(kc == 0), stop=(kc == DC - 1))
                            nc.vector.tensor_scalar_add(
                                out=dst[:, m, j * KB:(j + 1) * KB],
                                in0=psp[:], scalar1=b_fm[:, m:m + 1])

                # v: natural projection (lhsT = vT chunk, rhs = Wv)
                wv_sb = p1.tile([P, DC, D], f32r, tag="w_sb", name="wv_sb")
                nc.gpsimd.dma_start(
                    wv_sb[:], Wv.rearrange("(c p) n -> p c n", p=P))
                for j in range(S // KB):
                    fmr = tp.tile([P, DC, KB], f32r, tag="in_fm", name="v_fm")
                    for half in range(2):
                        nat2 = tpn.tile([P, 2, D], f32, tag="in_nat4",
                                        name="v_nat")
                        base = j * KB + half * 2 * P
                        nc.sync.dma_start(
                            nat2[:], v[base:base + 2 * P, :].rearrange(
                                "(t p) d -> p t d", p=P))
                        for rt in range(2):
                            _transpose_rows(nc, ps_t1, ident, nat2[:, rt, :],
                                            fmr, half * 2 + rt)
                    for rt in range(KB // P):
                        psv = ps_p.tile([P, D], f32, tag="ps_proj")
                        for kc in range(DC):
                            nc.tensor.matmul(
                                psv[:], lhsT=fmr[:, kc, rt * P:(rt + 1) * P],
                                rhs=wv_sb[:, kc, :],
                                start=(kc == 0), stop=(kc == DC - 1))
                        nc.vector.tensor_copy(
                            v_full[:, j * (KB // P) + rt, :], psv[:])

            # ---------- pass 2: attention + LN1, h -> DRAM ----------
            with ExitStack() as p2ctx:
                p2 = p2ctx.enter_context(tc.tile_pool(name="ph2", bufs=1))
                ptp = p2ctx.enter_context(tc.tile_pool(name="ptp", bufs=6))
                ps_o = p2ctx.enter_context(
                    tc.tile_pool(name="ps_o", bufs=4, space="PSUM"))
                ps_r = p2ctx.enter_context(
                    tc.tile_pool(name="ps_r", bufs=2, space="PSUM"))

                g1_bc = _bcast_load(nc, p2, g1[:], D, "g1")
                be1_bc = _bcast_load(nc, p2, be1[:], D, "be1")
                ht_early = []
                w1_sb = wp.tile([P, DC, F], f32r, tag="w1")
                nc.gpsimd.dma_start(
                    w1_sb[:], W1.rearrange("(c p) n -> p c n", p=P))
                b1_fm = _fm_load(nc, wp, b1[:], FC, "b1")

                for qb in range(NQB):
                    po = [ps_o.tile([P, D], f32, tag="ps_out", name=f"po{i}")
                          for i in range(4)]
                    rsum_sb = ep.tile([P, 4], f32, tag="rsum_sb")
                    for kc in range(SC):
                        if qb >= 2 and kc == 16:
                            htr = htp.tile([P, DC, QB], f32r, tag="ht_blk",
                                           name=f"ht{qb - 2}")
                            for qc in range(4):
                                _transpose_rows(
                                    nc, ps_a, ident,
                                    h_full[:, (qb - 2) * 4 + qc, :], htr, qc)
                            ht_early.append(htr)
                        pss = ps_a.tile([P, QB], f32, tag="ps_tp", name="pss")
                        for dc in range(DC):
                            nc.tensor.matmul(
                                pss[:], lhsT=kt_full[:, dc, kc * P:(kc + 1) * P],
                                rhs=qt_full[:, dc, qb * QB:(qb + 1) * QB],
                                start=(dc == 0), stop=(dc == DC - 1))
                        ptile = ptp.tile([P, QB], bf16, tag="pt")
                        nc.scalar.activation(
                            out=ptile[:], in_=pss[:],
                            func=mybir.ActivationFunctionType.Exp,
                            bias=0.0, scale=SCALE, alpha=0.0)
                        pr = ps_r.tile([P, 4, 4], f32, tag="ps_rsum")
                        for qc in range(4):
                            nc.tensor.matmul(
                                po[qc][:], lhsT=ptile[:, qc * P:(qc + 1) * P],
                                rhs=v_full[:, kc, :],
                                start=(kc == 0), stop=(kc == SC - 1))
                            nc.tensor.matmul(
                                pr[:, qc, :],
                                lhsT=ptile[:, qc * P:(qc + 1) * P],
                                rhs=ones_r[:],
                                start=True, stop=True)
                        if kc == 0:
                            nc.vector.tensor_copy(rsum_sb[:], pr[:, :, 0])
                        else:
                            nc.vector.tensor_add(out=rsum_sb[:],
                                                 in0=rsum_sb[:],
                                                 in1=pr[:, :, 0])
                    rinv = ep.tile([P, 4], f32, tag="rinv")
                    nc.vector.reciprocal(out=rinv[:], in_=rsum_sb[:])
                    for qc in range(4):
                        rc = qb * 4 + qc
                        t = h_full[:, rc, :]
                        nc.vector.tensor_scalar_mul(
                            out=t, in0=po[qc][:],
                            scalar1=rinv[:, qc:qc + 1])
                        xt = io.tile([P, D], f32, tag="in_nat", name="x_nat")
                        nc.sync.dma_start(xt[:], x[rc * P:(rc + 1) * P, :])
                        nc.vector.tensor_add(out=t, in0=t, in1=bv_bc[:])
                        nc.vector.tensor_add(out=t, in0=t, in1=xt[:])
                        _apply_ln(nc, ep, t, eps_t, g1_bc, be1_bc)

        # ---------- pass 3: FFN + LN2 ----------
        with ExitStack() as p3ctx:
            p3 = p3ctx.enter_context(tc.tile_pool(name="ph3", bufs=1))
            f1p = p3ctx.enter_context(tc.tile_pool(name="f1p", bufs=1))
            ps_f = p3ctx.enter_context(
                tc.tile_pool(name="ps_f", bufs=2, space="PSUM"))
            ps_g = p3ctx.enter_context(
                tc.tile_pool(name="ps_g", bufs=3, space="PSUM"))

            g2_bc = _bcast_load(nc, p3, g2[:], D, "g2")
            be2_bc = _bcast_load(nc, p3, be2[:], D, "be2")
            b2_bc = _bcast_load(nc, p3, b2[:], D, "b2")
            w2_sb = p3.tile([P, FC, D], f32r, tag="w2")
            nc.gpsimd.dma_start(
                w2_sb[:], W2.rearrange("(c p) n -> p c n", p=P))

            for fb in range(NQB):
                if fb < 2:
                    htr = ht_early[fb]
                else:
                    htr = htp.tile([P, DC, QB], f32r, tag="ht_blk",
                                   name=f"htl{fb}")
                    for qc in range(4):
                        _transpose_rows(nc, ps_a, ident,
                                        h_full[:, fb * 4 + qc, :], htr, qc)
                f1t = f1p.tile([P, FC, QB], f32r, tag="f1t")
                for fc in range(FC):
                    psf = ps_f.tile([P, QB], f32, tag="ps_ffn")
                    for dc in range(DC):
                        nc.tensor.matmul(
                            psf[:], lhsT=w1_sb[:, dc, fc * P:(fc + 1) * P],
                            rhs=htr[:, dc, :],
                            start=(dc == 0), stop=(dc == DC - 1))
                    nc.scalar.activation(
                        out=f1t[:, fc, :], in_=psf[:],
                        func=mybir.ActivationFunctionType.Relu,
                        bias=b1_fm[:, fc:fc + 1], scale=1.0, alpha=0.0)
                for qc in range(4):
                    rc = fb * 4 + qc
                    pso = ps_g.tile([P, D], f32, tag="ps_out2")
                    for fc in range(FC):
                        nc.tensor.matmul(
                            pso[:], lhsT=f1t[:, fc, qc * P:(qc + 1) * P],
                            rhs=w2_sb[:, fc, :],
                            start=(fc == 0), stop=(fc == FC - 1))
                    t = ep.tile([P, D], f32, tag="row_t", name="out_t")
                    nc.vector.tensor_add(out=t[:], in0=pso[:], in1=b2_bc[:])
                    nc.vector.tensor_add(out=t[:], in0=t[:],
                                         in1=h_full[:, rc, :])
                    _apply_ln(nc, ep, t[:], eps_t, g2_bc, be2_bc)
                    nc.sync.dma_start(out[rc * P:(rc + 1) * P, :], t[:])

    nc.finalize()
    return nc


_CACHE = {}
_LOCK = threading.Lock()


def _get_program():
    with _LOCK:
        if "nc" not in _CACHE:
            _CACHE["nc"] = build_program()
        return _CACHE["nc"]


def kernel(**inputs):
    nc = _get_program()
    weights = {n: np.ascontiguousarray(inputs[n]) for n in
               ["Wq", "bq", "Wk", "bk", "Wv", "bv", "gamma1", "beta1",
                "W1", "b1", "W2", "b2", "gamma2", "beta2"]}
    in_maps = []
    for c in range(N_CORES):
        b, h = c // 2, c % 2
        sl = slice(h * M, (h + 1) * M)
        in_maps.append({
            "q": np.ascontiguousarray(inputs["q"][b, sl]),
            "k": np.ascontiguousarray(inputs["k"][b]),
            "v": np.ascontiguousarray(inputs["v"][b]),
            "x": np.ascontiguousarray(inputs["x"][b, sl]),
            **weights,
        })
    res = run_bass_kernel_spmd(nc, in_maps, list(range(N_CORES)))
    out = np.empty((B, S, D), np.float32)
    for c in range(N_CORES):
        b, h = c // 2, c % 2
        out[b, h * M:(h + 1) * M] = res.results[c]["out"]
    return out



# revision 6
# speedup vs baseline: 1.4562x; 1.4562x over previous
"""Trainium2 Bass kernel for a single-head transformer encoder layer.

Problem shapes (hardcoded): B=4, S=4096, D=512, D_FFN=2048, fp32.
Sharding: 8 cores; core c handles batch b=c//2, query-row half h=c%2
(2048 q rows each). K/V for the batch's full sequence are handled
on-core (duplicated across the 2 cores sharing a batch).

v3 structure (fp8 attention via DoubleRow, bf16 FFN, host algebra):
  host: G = Wk@Wq^T folds both QKV score projections into one;
        abias = (k@(Wk@bq))*SCALE - C handles bq exactly (bk drops out
        of softmax); q/k uploaded pre-transposed bf16; v/W* bf16.
  pass 1: load kT -> fp8, load qT -> project by G (bf16) -> qgt fp8,
          load v natural -> fp8. No PE transposes at all.
  pass 2: per 512-q block: scoresT[k,q] via fp8 DoubleRow (contraction
          2x128/step); exp on ACT (bias=abias-C, scale=1/sqrt(D)) ->
          ptile fp8; Z^T[d,q] = v^T P via fp8 DoubleRow accumulating
          over all 32 k-chunks in 4 PSUM banks; row sums ride via a
          ones DoubleRow matmul into a 5th bank. Then attn = (Z@Wv)
          (bf16) scaled by 1/rsum on drain, +bv +x, LN1 -> h in SBUF.
  pass 3: FFN per 512-row block: PE-transpose h, FFN1 (relu+bias in
          ACT) bf16, FFN2 bf16, +b2 +h residual, LN2, store.
"""

import math
import threading
from contextlib import ExitStack

import ml_dtypes
import numpy as np

import concourse.bass as bass
import concourse.tile as tile
from concourse import bacc, mybir
from concourse.bass_utils import run_bass_kernel_spmd
from concourse.masks import make_identity

P = 128
B, S, D = 4, 4096, 512
F = 4 * D                    # 2048
M = S // 2                   # q rows per core
DC = D // P                  # 4 feature chunks
FC = F // P                  # 16 ffn chunks
SC = S // P                  # 32 k chunks
RC = M // P                  # 16 row chunks per core
QB = 512                     # q-block cols
NQB = M // QB                # 4
EPS = 1e-5
SCALE = 1.0 / math.sqrt(D)
CSHIFT = 2.0                 # exp shift; cancels in softmax normalization
f32 = mybir.dt.float32
bf16 = mybir.dt.bfloat16
fp8 = mybir.dt.float8e4
N_CORES = 8
DR = mybir.MatmulPerfMode.DoubleRow


def _ln_stats(nc, pool, t):
    fmax = nc.vector.BN_STATS_FMAX
    if D <= fmax:
        stats = pool.tile([P, nc.vector.BN_STATS_DIM], f32, tag="ln_stats")
        nc.vector.bn_stats(out=stats[:], in_=t[:])
        mv = pool.tile([P, nc.vector.BN_AGGR_DIM], f32, tag="ln_mv")
        nc.vector.bn_aggr(out=mv[:], in_=stats[:])
    else:
        sub = math.gcd(fmax, D)
        nsub = D // sub
        tr = t.rearrange("p (n s) -> p n s", s=sub)
        stats = pool.tile([P, nsub, nc.vector.BN_STATS_DIM], f32, tag="ln_stats")
        for i in range(nsub):
            nc.vector.bn_stats(out=stats[:, i, :], in_=tr[:, i, :])
        mv = pool.tile([P, nc.vector.BN_AGGR_DIM], f32, tag="ln_mv")
        nc.vector.bn_aggr(out=mv[:], in_=stats[:])
    return mv[:, 0:1], mv[:, 1:2]


def _apply_ln(nc, pool, t, eps_t, gamma_bc, beta_bc):
    mean, var = _ln_stats(nc, pool, t)
    nc.scalar.activation(out=var, in_=var,
                         func=mybir.ActivationFunctionType.Sqrt,
                         bias=eps_t[:], scale=1.0, alpha=0.0)
    nc.vector.reciprocal(out=var, in_=var)
    nc.vector.tensor_scalar(out=t[:], in0=t[:], scalar1=mean, scalar2=var,
                            op0=mybir.AluOpType.subtract,
                            op1=mybir.AluOpType.mult)
    nc.vector.tensor_mul(out=t[:], in0=t[:], in1=gamma_bc[:])
    nc.vector.tensor_add(out=t[:], in0=t[:], in1=beta_bc[:])


def _bcast_load(nc, pool, vec_ap, n, tag):
    t = pool.tile([P, n], f32, tag=tag)
    src = bass.AP(tensor=vec_ap.tensor, offset=vec_ap.offset,
                  ap=[[0, P]] + list(vec_ap.ap))
    nc.gpsimd.dma_start(out=t[:], in_=src)
    return t


def _fm_load(nc, pool, vec_ap, chunks, tag):
    t = pool.tile([P, chunks], f32, tag=tag)
    nc.sync.dma_start(t[:], vec_ap.rearrange("(c p) -> p c", p=P))
    return t


def _transpose_rows(nc, ps_pool, ident, nat, fm, rt):
    """PE-transpose nat [P,512] into fm[:, :, rt*P:(rt+1)*P] via one
    4-quadrant PSUM bank and a single batched DVE drain."""
    pst = ps_pool.tile([P, DC, P], f32, tag="ps_tp")
    for dc in range(DC):
        nc.tensor.transpose(pst[:, dc, :], nat[:, dc * P:(dc + 1) * P], ident)
    nc.vector.tensor_copy(fm[:, :, rt * P:(rt + 1) * P], pst[:])


def build_program():
    nc = bacc.Bacc()
    qT = nc.dram_tensor("qT", [D, M], bf16, kind="ExternalInput")
    kT = nc.dram_tensor("kT", [D, S], bf16, kind="ExternalInput")
    v = nc.dram_tensor("v", [S, D], bf16, kind="ExternalInput")
    x = nc.dram_tensor("x", [M, D], f32, kind="ExternalInput")
    G = nc.dram_tensor("G", [D, D], bf16, kind="ExternalInput")
    Wv = nc.dram_tensor("Wv", [D, D], bf16, kind="ExternalInput")
    W1 = nc.dram_tensor("W1", [D, F], bf16, kind="ExternalInput")
    W2 = nc.dram_tensor("W2", [F, D], bf16, kind="ExternalInput")
    abias = nc.dram_tensor("abias", [S], f32, kind="ExternalInput")
    bv = nc.dram_tensor("bv", [D], f32, kind="ExternalInput")
    b1 = nc.dram_tensor("b1", [F], f32, kind="ExternalInput")
    b2 = nc.dram_tensor("b2", [D], f32, kind="ExternalInput")
    g1 = nc.dram_tensor("gamma1", [D], f32, kind="ExternalInput")
    be1 = nc.dram_tensor("beta1", [D], f32, kind="ExternalInput")
    g2 = nc.dram_tensor("gamma2", [D], f32, kind="ExternalInput")
    be2 = nc.dram_tensor("beta2", [D], f32, kind="ExternalInput")
    out = nc.dram_tensor("out", [M, D], f32, kind="ExternalOutput")

    with tile.TileContext(nc) as tc, ExitStack() as ctx:
        g_pool = ctx.enter_context(tc.tile_pool(name="glob", bufs=1))
        io = ctx.enter_context(tc.tile_pool(name="io", bufs=3))
        htp = ctx.enter_context(tc.tile_pool(name="htp", bufs=2))
        wp = ctx.enter_context(tc.tile_pool(name="wp", bufs=1))
        ep = ctx.enter_context(tc.tile_pool(name="ep", bufs=2))

        ident_t = g_pool.tile([P, P], f32, tag="ident")
        make_identity(nc, ident_t[:])
        ident = ident_t[:]
        ones8 = g_pool.tile([P, 2, 4], fp8, tag="ones8")
        nc.vector.memset(ones8[:], 1.0)
        eps_t = g_pool.tile([P, 1], f32, tag="eps")
        nc.vector.memset(eps_t[:], EPS)
        h_full = g_pool.tile([P, RC, D], f32, tag="h_full")

        # FFN + Wv weights: straight bf16 loads, overlap with pass 1/2
        wv_sb = wp.tile([P, DC, D], bf16, tag="wv")
        nc.gpsimd.dma_start(wv_sb[:], Wv.rearrange("(c p) n -> p c n", p=P))
        w1_sb = wp.tile([P, DC, F], bf16, tag="w1")
        nc.gpsimd.dma_start(w1_sb[:], W1.rearrange("(c p) n -> p c n", p=P))
        w2_sb = wp.tile([P, FC, D], bf16, tag="w2")
        nc.gpsimd.dma_start(w2_sb[:], W2.rearrange("(c p) n -> p c n", p=P))
        b1_fm = _fm_load(nc, wp, b1[:], FC, "b1")
        abias_fm = _fm_load(nc, wp, abias[:], SC, "abias")

        with ExitStack() as actx:
            attn = actx.enter_context(tc.tile_pool(name="attn", bufs=1))
            kt8 = attn.tile([P, DC, S], fp8, tag="kt8")
            qgt = attn.tile([P, DC, M], fp8, tag="qgt")
            v8 = attn.tile([P, SC, D], fp8, tag="v8")

            # ---------- pass 1: load + cast + q-side G projection ----------
            with ExitStack() as p1ctx:
                p1 = p1ctx.enter_context(tc.tile_pool(name="ph1", bufs=1))
                stg = p1ctx.enter_context(tc.tile_pool(name="stg", bufs=2))
                ps_p = p1ctx.enter_context(
                    tc.tile_pool(name="ps_p", bufs=3, space="PSUM"))

                g_sb = p1.tile([P, DC, D], bf16, tag="g_sb")
                nc.sync.dma_start(g_sb[:], G.rearrange("(c p) n -> p c n", p=P))
                qt_raw = p1.tile([P, DC, M], bf16, tag="qt_raw")
                nc.sync.dma_start(qt_raw[:],
                                  qT.rearrange("(c p) r -> p c r", p=P))

                # kT: stage per feature-chunk, cast bf16 -> fp8 on ACT
                for c in range(DC):
                    kstg = stg.tile([P, S], bf16, tag="kstg")
                    nc.sync.dma_start(kstg[:], kT[c * P:(c + 1) * P, :])
                    nc.scalar.activation(
                        out=kt8[:, c, :], in_=kstg[:],
                        func=mybir.ActivationFunctionType.Copy,
                        bias=0.0, scale=1.0, alpha=0.0)

                # v natural: stage 8 chunks, cast on DVE
                vr = v.rearrange("(c p) d -> p c d", p=P)
                for j in range(8):
                    vstg = stg.tile([P, 4, D], bf16, tag="vstg")
                    nc.sync.dma_start(vstg[:], vr[:, j * 4:(j + 1) * 4, :])
                    nc.vector.tensor_copy(v8[:, j * 4:(j + 1) * 4, :], vstg[:])

                # qgt = G @ qT  (bf16 matmuls, fp8 drains on DVE)
                for j in range(NQB):
                    for m in range(DC):
                        psp = ps_p.tile([P, QB], f32, tag="ps_proj")
                        for c in range(DC):
                            nc.tensor.matmul(
                                psp[:], lhsT=g_sb[:, c, m * P:(m + 1) * P],
                                rhs=qt_raw[:, c, j * QB:(j + 1) * QB],
                                start=(c == 0), stop=(c == DC - 1))
                        nc.vector.tensor_copy(
                            qgt[:, m, j * QB:(j + 1) * QB], psp[:])

            # ---------- pass 2: attention + LN1 ----------
            with ExitStack() as p2ctx:
                p2 = p2ctx.enter_context(tc.tile_pool(name="ph2", bufs=1))
                ptp = p2ctx.enter_context(tc.tile_pool(name="ptp", bufs=3))
                zsp = p2ctx.enter_context(tc.tile_pool(name="zsp", bufs=2))
                ps_s = p2ctx.enter_context(
                    tc.tile_pool(name="ps_s", bufs=3, space="PSUM"))
                ps_z = p2ctx.enter_context(
                    tc.tile_pool(name="ps_z", bufs=1, space="PSUM"))
                ps_r = p2ctx.enter_context(
                    tc.tile_pool(name="ps_r", bufs=1, space="PSUM"))

                g1_bc = _bcast_load(nc, p2, g1[:], D, "g1")
                be1_bc = _bcast_load(nc, p2, be1[:], D, "be1")
                bv_bc = _bcast_load(nc, p2, bv[:], D, "bv")

                for qb in range(NQB):
                    psZ = ps_z.tile([P, DC, QB], f32, tag="psZ")
                    pr = ps_r.tile([P, 4, 4], f32, tag="pr")
                    ptile = None
                    for kc in range(SC):
                        pss = ps_s.tile([P, QB], f32, tag="pss")
                        for d2 in (0, 2):
                            nc.tensor.matmul(
                                pss[:],
                                lhsT=kt8[:, d2:d2 + 2, kc * P:(kc + 1) * P],
                                rhs=qgt[:, d2:d2 + 2, qb * QB:(qb + 1) * QB],
                                start=(d2 == 0), stop=(d2 == 2),
                                perf_mode=DR)
                        if kc % 2 == 0:
                            ptile = ptp.tile([P, 2, QB], fp8, tag="pt")
                        nc.scalar.activation(
                            out=ptile[:, kc % 2, :], in_=pss[:],
                            func=mybir.ActivationFunctionType.Exp,
                            bias=abias_fm[:, kc:kc + 1], scale=SCALE,
                            alpha=0.0)
                        if kc % 2 == 1:
                            kp = kc // 2
                            for dc in range(DC):
                                nc.tensor.matmul(
                                    psZ[:, dc, :],
                                    lhsT=v8[:, kc - 1:kc + 1,
                                            dc * P:(dc + 1) * P],
                                    rhs=ptile[:],
                                    start=(kp == 0), stop=(kp == SC // 2 - 1),
                                    perf_mode=DR)
                            for qc in range(4):
                                nc.tensor.matmul(
                                    pr[:, qc, :],
                                    lhsT=ptile[:, :, qc * P:(qc + 1) * P],
                                    rhs=ones8[:],
                                    start=(kp == 0), stop=(kp == SC // 2 - 1),
                                    perf_mode=DR)
                    rsum_sb = ep.tile([P, 4], f32, tag="rsum_sb")
                    nc.vector.tensor_copy(rsum_sb[:], pr[:, :, 0])
                    rinv = ep.tile([P, 4], f32, tag="rinv")
                    nc.vector.reciprocal(out=rinv[:], in_=rsum_sb[:])
                    z_sb = zsp.tile([P, DC, QB], bf16, tag="z_sb")
                    nc.vector.tensor_copy(z_sb[:], psZ[:])
                    for qc in range(4):
                        psA = ps_s.tile([P, QB], f32, tag="pss", name="psA")
                        for dc in range(DC):
                            nc.tensor.matmul(
                                psA[:],
                                lhsT=z_sb[:, dc, qc * P:(qc + 1) * P],
                                rhs=wv_sb[:, dc, :],
                                start=(dc == 0), stop=(dc == DC - 1))
                        rc = qb * 4 + qc
                        t = h_full[:, rc, :]
                        nc.vector.tensor_scalar_mul(
                            out=t, in0=psA[:], scalar1=rinv[:, qc:qc + 1])
                        xt = io.tile([P, D], f32, tag="in_nat", name="x_nat")
                        nc.sync.dma_start(xt[:], x[rc * P:(rc + 1) * P, :])
                        nc.vector.tensor_add(out=t, in0=t, in1=bv_bc[:])
                        nc.vector.tensor_add(out=t, in0=t, in1=xt[:])
                        _apply_ln(nc, ep, t, eps_t, g1_bc, be1_bc)

        # ---------- pass 3: FFN + LN2 ----------
        with ExitStack() as p3ctx:
            p3 = p3ctx.enter_context(tc.tile_pool(name="ph3", bufs=1))
            f1p = p3ctx.enter_context(tc.tile_pool(name="f1p", bufs=1))
            ps_f = p3ctx.enter_context(
                tc.tile_pool(name="ps_f", bufs=2, space="PSUM"))
            ps_g = p3ctx.enter_context(
                tc.tile_pool(name="ps_g", bufs=3, space="PSUM"))
            ps_t = p3ctx.enter_context(
                tc.tile_pool(name="ps_t", bufs=2, space="PSUM"))

            g2_bc = _bcast_load(nc, p3, g2[:], D, "g2")
            be2_bc = _bcast_load(nc, p3, be2[:], D, "be2")
            b2_bc = _bcast_load(nc, p3, b2[:], D, "b2")

            def _transpose_h(fb):
                htr = htp.tile([P, DC, QB], bf16, tag="ht_blk",
                               name=f"htl{fb}")
                for qc in range(4):
                    _transpose_rows(nc, ps_t, ident,
                                    h_full[:, fb * 4 + qc, :], htr, qc)
                return htr

            ht_next = _transpose_h(0)
            for fb in range(NQB):
                htr = ht_next
                f1t = f1p.tile([P, FC, QB], bf16, tag="f1t")
                for fc in range(FC):
                    psf = ps_f.tile([P, QB], f32, tag="ps_ffn")
                    for dc in range(DC):
                        nc.tensor.matmul(
                            psf[:], lhsT=w1_sb[:, dc, fc * P:(fc + 1) * P],
                            rhs=htr[:, dc, :],
                            start=(dc == 0), stop=(dc == DC - 1))
                    nc.scalar.activation(
                        out=f1t[:, fc, :], in_=psf[:],
                        func=mybir.ActivationFunctionType.Relu,
                        bias=b1_fm[:, fc:fc + 1], scale=1.0, alpha=0.0)
                if fb + 1 < NQB:
                    ht_next = _transpose_h(fb + 1)
                for qc in range(4):
                    rc = fb * 4 + qc
                    pso = ps_g.tile([P, D], f32, tag="ps_out2")
                    for fc in range(FC):
                        nc.tensor.matmul(
                            pso[:], lhsT=f1t[:, fc, qc * P:(qc + 1) * P],
                            rhs=w2_sb[:, fc, :],
                            start=(fc == 0), stop=(fc == FC - 1))
                    t = ep.tile([P, D], f32, tag="row_t", name="out_t")
                    nc.vector.tensor_add(out=t[:], in0=pso[:], in1=b2_bc[:])
                    nc.vector.tensor_add(out=t[:], in0=t[:],
                                         in1=h_full[:, rc, :])
                    _apply_ln(nc, ep, t[:], eps_t, g2_bc, be2_bc)
                    nc.sync.dma_start(out[rc * P:(rc + 1) * P, :], t[:])

    nc.finalize()
    return nc


_CACHE = {}
_LOCK = threading.Lock()


def _get_program():
    with _LOCK:
        if "nc" not in _CACHE:
            _CACHE["nc"] = build_program()
        return _CACHE["nc"]


def make_in_maps(inputs):
    bf = ml_dtypes.bfloat16
    # device projection applies G_dev^T to q^T, so upload Wq@Wk^T to get
    # scoresT = k (Wk Wq^T) q^T = K Q^T
    G = (inputs["Wq"].astype(np.float32)
         @ inputs["Wk"].astype(np.float32).T).astype(bf)
    wkbq = inputs["Wk"].astype(np.float32) @ inputs["bq"].astype(np.float32)
    weights = {
        "G": np.ascontiguousarray(G),
        "Wv": np.ascontiguousarray(inputs["Wv"].astype(bf)),
        "W1": np.ascontiguousarray(inputs["W1"].astype(bf)),
        "W2": np.ascontiguousarray(inputs["W2"].astype(bf)),
        "bv": np.ascontiguousarray(inputs["bv"].astype(np.float32)),
        "b1": np.ascontiguousarray(inputs["b1"].astype(np.float32)),
        "b2": np.ascontiguousarray(inputs["b2"].astype(np.float32)),
        "gamma1": np.ascontiguousarray(inputs["gamma1"].astype(np.float32)),
        "beta1": np.ascontiguousarray(inputs["beta1"].astype(np.float32)),
        "gamma2": np.ascontiguousarray(inputs["gamma2"].astype(np.float32)),
        "beta2": np.ascontiguousarray(inputs["beta2"].astype(np.float32)),
    }
    in_maps = []
    for c in range(N_CORES):
        b, h = c // 2, c % 2
        sl = slice(h * M, (h + 1) * M)
        kb = inputs["k"][b].astype(np.float32)
        abias = (kb @ wkbq) * SCALE - CSHIFT
        in_maps.append({
            "qT": np.ascontiguousarray(inputs["q"][b, sl].T.astype(bf)),
            "kT": np.ascontiguousarray(kb.T.astype(bf)),
            "v": np.ascontiguousarray(inputs["v"][b].astype(bf)),
            "x": np.ascontiguousarray(inputs["x"][b, sl].astype(np.float32)),
            "abias": np.ascontiguousarray(abias.astype(np.float32)),
            **weights,
        })
    return in_maps


def kernel(**inputs):
    nc = _get_program()
    in_maps = make_in_maps(inputs)
    res = run_bass_kernel_spmd(nc, in_maps, list(range(N_CORES)))
    out = np.empty((B, S, D), np.float32)
    for c in range(N_CORES):
        b, h = c // 2, c % 2
        out[b, h * M:(h + 1) * M] = res.results[c]["out"]
    return out


# revision 7
# speedup vs baseline: 1.6720x; 1.1482x over previous
"""Trainium2 Bass kernel for a single-head transformer encoder layer.

Problem shapes (hardcoded): B=4, S=4096, D=512, D_FFN=2048, fp32.
Sharding: 8 cores; core c handles batch b=c//2, query-row half h=c%2
(2048 q rows each). K/V for the batch's full sequence are handled
on-core (duplicated across the 2 cores sharing a batch).

v4 structure (fp8 attention via DoubleRow, bf16 FFN, host algebra):
  host: G = Wq@Wk^T (applied transposed on device) folds both score
        projections into one; exp bias handles bq exactly (bk drops
        out of softmax); q/k uploaded pre-transposed bf16; v/W* bf16.
  pass 1: kT -> fp8 (ACT casts); qT -> fp8 -> project by G (fp8
          DoubleRow) -> qgt fp8; v natural -> fp8. No PE transposes.
  pass 2: per 512-q block: scoresT[k,q] via fp8 DoubleRow; exp on ACT
          (shift C cancels in normalization) -> ptile fp8; Z^T = v^T P
          fp8 DoubleRow accumulated over 32 k-chunks in 4 PSUM banks;
          row sums via ones DoubleRow matmul in a 5th bank.
          attn = (Z@Wv) fp8 DoubleRow, scaled 1/rsum on drain, +x.
          LN1 deferred to a batched pass (one sqrt -> no ACT table
          thrash against exp).
  pass 3: FFN per 512-row block: PE-transpose h, FFN1 (relu+bias in
          ACT) bf16, FFN2 bf16, +h residual, batched LN2, store.
  Ops for zero biases / unit gamma / zero beta are skipped when the
  runtime values allow (general path kept otherwise).
"""

import math
import threading
from contextlib import ExitStack

import ml_dtypes
import numpy as np

import concourse.bass as bass
import concourse.tile as tile
from concourse import bacc, mybir
from concourse.bass_utils import run_bass_kernel_spmd
from concourse.masks import make_identity

P = 128
B, S, D = 4, 4096, 512
F = 4 * D                    # 2048
M = S // 2                   # q rows per core
DC = D // P                  # 4 feature chunks
FC = F // P                  # 16 ffn chunks
SC = S // P                  # 32 k chunks
RC = M // P                  # 16 row chunks per core
QB = 512                     # q-block cols
NQB = M // QB                # 4
EPS = 1e-5
SCALE = 1.0 / math.sqrt(D)
CSHIFT = 2.5                 # exp shift; cancels in softmax normalization
f32 = mybir.dt.float32
bf16 = mybir.dt.bfloat16
fp8 = mybir.dt.float8e4
N_CORES = 8
DR = mybir.MatmulPerfMode.DoubleRow


def _bcast_load(nc, pool, vec_ap, n, tag):
    t = pool.tile([P, n], f32, tag=tag)
    src = bass.AP(tensor=vec_ap.tensor, offset=vec_ap.offset,
                  ap=[[0, P]] + list(vec_ap.ap))
    nc.gpsimd.dma_start(out=t[:], in_=src)
    return t


def _fm_load(nc, pool, vec_ap, chunks, tag):
    t = pool.tile([P, chunks], f32, tag=tag)
    nc.sync.dma_start(t[:], vec_ap.rearrange("(c p) -> p c", p=P))
    return t


def _transpose_rows(nc, ps_pool, ident, nat, fm, rt):
    pst = ps_pool.tile([P, DC, P], f32, tag="ps_tp")
    for dc in range(DC):
        nc.tensor.transpose(pst[:, dc, :], nat[:, dc * P:(dc + 1) * P], ident)
    nc.vector.tensor_copy(fm[:, :, rt * P:(rt + 1) * P], pst[:])


def build_program(spec):
    """spec: frozenset of flags among {'bv0','b20','bq0','ln1_triv',
    'ln2_triv'} marking inputs that are exactly zero / identity."""
    nc = bacc.Bacc()
    qT = nc.dram_tensor("qT", [D, M], bf16, kind="ExternalInput")
    kT = nc.dram_tensor("kT", [D, S], bf16, kind="ExternalInput")
    v = nc.dram_tensor("v", [S, D], bf16, kind="ExternalInput")
    x = nc.dram_tensor("x", [M, D], f32, kind="ExternalInput")
    G = nc.dram_tensor("G", [D, D], bf16, kind="ExternalInput")
    Wv = nc.dram_tensor("Wv", [D, D], bf16, kind="ExternalInput")
    W1 = nc.dram_tensor("W1", [D, F], bf16, kind="ExternalInput")
    W2 = nc.dram_tensor("W2", [F, D], bf16, kind="ExternalInput")
    b1 = nc.dram_tensor("b1", [F], f32, kind="ExternalInput")
    abias = (None if "bq0" in spec else
             nc.dram_tensor("abias", [S], f32, kind="ExternalInput"))
    bv = (None if "bv0" in spec else
          nc.dram_tensor("bv", [D], f32, kind="ExternalInput"))
    b2 = (None if "b20" in spec else
          nc.dram_tensor("b2", [D], f32, kind="ExternalInput"))
    if "ln1_triv" not in spec:
        g1 = nc.dram_tensor("gamma1", [D], f32, kind="ExternalInput")
        be1 = nc.dram_tensor("beta1", [D], f32, kind="ExternalInput")
    if "ln2_triv" not in spec:
        g2 = nc.dram_tensor("gamma2", [D], f32, kind="ExternalInput")
        be2 = nc.dram_tensor("beta2", [D], f32, kind="ExternalInput")
    out = nc.dram_tensor("out", [M, D], f32, kind="ExternalOutput")

    with tile.TileContext(nc) as tc, ExitStack() as ctx:
        g_pool = ctx.enter_context(tc.tile_pool(name="glob", bufs=1))
        io = ctx.enter_context(tc.tile_pool(name="io", bufs=3))
        htp = ctx.enter_context(tc.tile_pool(name="htp", bufs=2))
        wp = ctx.enter_context(tc.tile_pool(name="wp", bufs=1))
        ep = ctx.enter_context(tc.tile_pool(name="ep", bufs=2))

        ident_t = g_pool.tile([P, P], f32, tag="ident")
        make_identity(nc, ident_t[:])
        ident = ident_t[:]
        ones8 = g_pool.tile([P, 2, 4], fp8, tag="ones8")
        nc.vector.memset(ones8[:], 1.0)
        eps_t = g_pool.tile([P, 1], f32, tag="eps")
        nc.vector.memset(eps_t[:], EPS)
        negc_t = g_pool.tile([P, 1], f32, tag="negc")
        nc.vector.memset(negc_t[:], -CSHIFT)
        h_full = g_pool.tile([P, RC, D], f32, tag="h_full")

        def ln_batch(pool, rows, gamma_bc, beta_bc):
            """Batched LN over a list of [P, D] APs: one sqrt call."""
            n = len(rows)
            mvs = pool.tile([P, n, 2], f32, tag="ln_mvs")
            for i, t in enumerate(rows):
                stats = pool.tile([P, 6], f32, tag="ln_stats")
                nc.vector.bn_stats(out=stats[:], in_=t)
                nc.vector.bn_aggr(out=mvs[:, i, :], in_=stats[:])
            nc.scalar.activation(out=mvs[:, :, 1:2], in_=mvs[:, :, 1:2],
                                 func=mybir.ActivationFunctionType.Sqrt,
                                 bias=eps_t[:], scale=1.0, alpha=0.0)
            nc.vector.reciprocal(out=mvs[:, :, 1], in_=mvs[:, :, 1])
            for i, t in enumerate(rows):
                nc.vector.tensor_scalar(
                    out=t, in0=t, scalar1=mvs[:, i, 0:1],
                    scalar2=mvs[:, i, 1:2],
                    op0=mybir.AluOpType.subtract,
                    op1=mybir.AluOpType.mult)
                if gamma_bc is not None:
                    nc.vector.tensor_mul(out=t, in0=t, in1=gamma_bc[:])
                    nc.vector.tensor_add(out=t, in0=t, in1=beta_bc[:])

        # FFN + Wv weights: straight bf16 loads, overlap with pass 1/2
        wv_sb = wp.tile([P, DC, D], bf16, tag="wv")
        nc.gpsimd.dma_start(wv_sb[:], Wv.rearrange("(c p) n -> p c n", p=P))
        w1_sb = wp.tile([P, DC, F], bf16, tag="w1")
        nc.gpsimd.dma_start(w1_sb[:], W1.rearrange("(c p) n -> p c n", p=P))
        w2_sb = wp.tile([P, FC, D], bf16, tag="w2")
        nc.gpsimd.dma_start(w2_sb[:], W2.rearrange("(c p) n -> p c n", p=P))
        b1_fm = _fm_load(nc, wp, b1[:], FC, "b1")
        wv8 = wp.tile([P, DC, D], fp8, tag="wv8")
        nc.vector.tensor_copy(wv8[:], wv_sb[:])
        abias_fm = (None if abias is None else
                    _fm_load(nc, wp, abias[:], SC, "abias"))

        with ExitStack() as actx:
            attn = actx.enter_context(tc.tile_pool(name="attn", bufs=1))
            kt8 = attn.tile([P, DC, S], fp8, tag="kt8")
            qgt = attn.tile([P, DC, M], fp8, tag="qgt")
            v8 = attn.tile([P, SC, D], fp8, tag="v8")

            # ---------- pass 1: load + cast + q-side G projection ----------
            with ExitStack() as p1ctx:
                p1 = p1ctx.enter_context(tc.tile_pool(name="ph1", bufs=1))
                stg = p1ctx.enter_context(tc.tile_pool(name="stg", bufs=2))
                qstg = p1ctx.enter_context(tc.tile_pool(name="qstg", bufs=2))
                ps_p = p1ctx.enter_context(
                    tc.tile_pool(name="ps_p", bufs=3, space="PSUM"))

                g_sb = p1.tile([P, DC, D], bf16, tag="g_sb")
                nc.sync.dma_start(g_sb[:], G.rearrange("(c p) n -> p c n", p=P))
                g8 = p1.tile([P, DC, D], fp8, tag="g8")
                nc.vector.tensor_copy(g8[:], g_sb[:])

                # kT chunks early (scores critical path); casts on ACT
                kTr = kT.rearrange("(c p) r -> p c r", p=P)
                for c in range(DC):
                    kstg = stg.tile([P, S], bf16, tag="kstg")
                    nc.sync.dma_start(kstg[:], kTr[:, c, :])
                    nc.scalar.activation(
                        out=kt8[:, c, :], in_=kstg[:],
                        func=mybir.ActivationFunctionType.Copy,
                        bias=0.0, scale=1.0, alpha=0.0)

                # qT per block: load, cast fp8, project by G (fp8 DR)
                qTr = qT.rearrange("(c p) r -> p c r", p=P)
                for j in range(NQB):
                    qstg_t = qstg.tile([P, DC, QB], bf16, tag="qstg")
                    nc.sync.dma_start(qstg_t[:],
                                      qTr[:, :, j * QB:(j + 1) * QB])
                    qt8 = qstg.tile([P, DC, QB], fp8, tag="qt8")
                    nc.vector.tensor_copy(qt8[:], qstg_t[:])
                    for m in range(DC):
                        psp = ps_p.tile([P, QB], f32, tag="ps_proj")
                        for c2 in (0, 2):
                            nc.tensor.matmul(
                                psp[:],
                                lhsT=g8[:, c2:c2 + 2, m * P:(m + 1) * P],
                                rhs=qt8[:, c2:c2 + 2, :],
                                start=(c2 == 0), stop=(c2 == 2),
                                perf_mode=DR)
                        nc.vector.tensor_copy(
                            qgt[:, m, j * QB:(j + 1) * QB], psp[:])

                # v natural: stage 8 chunks, cast on DVE
                vr = v.rearrange("(c p) d -> p c d", p=P)
                for j in range(8):
                    vstg = stg.tile([P, 4, D], bf16, tag="vstg")
                    nc.sync.dma_start(vstg[:], vr[:, j * 4:(j + 1) * 4, :])
                    nc.vector.tensor_copy(v8[:, j * 4:(j + 1) * 4, :], vstg[:])

            # ---------- pass 2: attention, LN1 deferred ----------
            with ExitStack() as p2ctx:
                p2 = p2ctx.enter_context(tc.tile_pool(name="ph2", bufs=1))
                ptp = p2ctx.enter_context(tc.tile_pool(name="ptp", bufs=3))
                zsp = p2ctx.enter_context(tc.tile_pool(name="zsp", bufs=2))
                ps_s = p2ctx.enter_context(
                    tc.tile_pool(name="ps_s", bufs=3, space="PSUM"))
                ps_z = p2ctx.enter_context(
                    tc.tile_pool(name="ps_z", bufs=1, space="PSUM"))
                ps_r = p2ctx.enter_context(
                    tc.tile_pool(name="ps_r", bufs=1, space="PSUM"))

                bv_bc = (None if bv is None else
                         _bcast_load(nc, p2, bv[:], D, "bv"))

                for qb in range(NQB):
                    psZ = ps_z.tile([P, DC, QB], f32, tag="psZ")
                    pr = ps_r.tile([P, 4, 4], f32, tag="pr")
                    ptile = None
                    for kc in range(SC):
                        pss = ps_s.tile([P, QB], f32, tag="pss")
                        for d2 in (0, 2):
                            nc.tensor.matmul(
                                pss[:],
                                lhsT=kt8[:, d2:d2 + 2, kc * P:(kc + 1) * P],
                                rhs=qgt[:, d2:d2 + 2, qb * QB:(qb + 1) * QB],
                                start=(d2 == 0), stop=(d2 == 2),
                                perf_mode=DR)
                        if kc % 2 == 0:
                            ptile = ptp.tile([P, 2, QB], fp8, tag="pt")
                        ebias = (negc_t[:] if abias_fm is None
                                 else abias_fm[:, kc:kc + 1])
                        nc.scalar.activation(
                            out=ptile[:, kc % 2, :], in_=pss[:],
                            func=mybir.ActivationFunctionType.Exp,
                            bias=ebias, scale=SCALE, alpha=0.0)
                        if kc % 2 == 1:
                            kp = kc // 2
                            for dc in range(DC):
                                nc.tensor.matmul(
                                    psZ[:, dc, :],
                                    lhsT=v8[:, kc - 1:kc + 1,
                                            dc * P:(dc + 1) * P],
                                    rhs=ptile[:],
                                    start=(kp == 0), stop=(kp == SC // 2 - 1),
                                    perf_mode=DR)
                            for qc in range(4):
                                nc.tensor.matmul(
                                    pr[:, qc, :],
                                    lhsT=ptile[:, :, qc * P:(qc + 1) * P],
                                    rhs=ones8[:],
                                    start=(kp == 0), stop=(kp == SC // 2 - 1),
                                    perf_mode=DR)
                    rsum_sb = ep.tile([P, 4], f32, tag="rsum_sb")
                    nc.vector.tensor_copy(rsum_sb[:], pr[:, :, 0])
                    rinv = ep.tile([P, 4], f32, tag="rinv")
                    nc.vector.reciprocal(out=rinv[:], in_=rsum_sb[:])
                    z8 = zsp.tile([P, DC, QB], fp8, tag="z8")
                    nc.vector.tensor_copy(z8[:], psZ[:])
                    for qc in range(4):
                        psA = ps_s.tile([P, QB], f32, tag="pss", name="psA")
                        for c2 in (0, 2):
                            nc.tensor.matmul(
                                psA[:],
                                lhsT=z8[:, c2:c2 + 2, qc * P:(qc + 1) * P],
                                rhs=wv8[:, c2:c2 + 2, :],
                                start=(c2 == 0), stop=(c2 == 2),
                                perf_mode=DR)
                        rc = qb * 4 + qc
                        t = h_full[:, rc, :]
                        nc.vector.tensor_scalar_mul(
                            out=t, in0=psA[:], scalar1=rinv[:, qc:qc + 1])
                        xt = io.tile([P, D], f32, tag="in_nat", name="x_nat")
                        nc.sync.dma_start(xt[:], x[rc * P:(rc + 1) * P, :])
                        if bv_bc is not None:
                            nc.vector.tensor_add(out=t, in0=t, in1=bv_bc[:])
                        nc.vector.tensor_add(out=t, in0=t, in1=xt[:])

                # pass 2.5: batched LN1 (single sqrt)
                if "ln1_triv" in spec:
                    g1_bc = be1_bc = None
                else:
                    g1_bc = _bcast_load(nc, p2, g1[:], D, "g1")
                    be1_bc = _bcast_load(nc, p2, be1[:], D, "be1")
                ln_batch(ep, [h_full[:, rc, :] for rc in range(RC)],
                         g1_bc, be1_bc)

        # ---------- pass 3: FFN + LN2 ----------
        with ExitStack() as p3ctx:
            p3 = p3ctx.enter_context(tc.tile_pool(name="ph3", bufs=1))
            f1p = p3ctx.enter_context(tc.tile_pool(name="f1p", bufs=1))
            ps_f = p3ctx.enter_context(
                tc.tile_pool(name="ps_f", bufs=2, space="PSUM"))
            ps_g = p3ctx.enter_context(
                tc.tile_pool(name="ps_g", bufs=3, space="PSUM"))
            ps_t = p3ctx.enter_context(
                tc.tile_pool(name="ps_t", bufs=2, space="PSUM"))

            if "ln2_triv" in spec:
                g2_bc = be2_bc = None
            else:
                g2_bc = _bcast_load(nc, p3, g2[:], D, "g2")
                be2_bc = _bcast_load(nc, p3, be2[:], D, "be2")
            b2_bc = (None if b2 is None else
                     _bcast_load(nc, p3, b2[:], D, "b2"))

            def _transpose_h(fb):
                htr = htp.tile([P, DC, QB], bf16, tag="ht_blk",
                               name=f"htl{fb}")
                for qc in range(4):
                    _transpose_rows(nc, ps_t, ident,
                                    h_full[:, fb * 4 + qc, :], htr, qc)
                return htr

            ht_next = _transpose_h(0)
            for fb in range(NQB):
                htr = ht_next
                f1t = f1p.tile([P, FC, QB], bf16, tag="f1t")
                for fc in range(FC):
                    psf = ps_f.tile([P, QB], f32, tag="ps_ffn")
                    for dc in range(DC):
                        nc.tensor.matmul(
                            psf[:], lhsT=w1_sb[:, dc, fc * P:(fc + 1) * P],
                            rhs=htr[:, dc, :],
                            start=(dc == 0), stop=(dc == DC - 1))
                    nc.scalar.activation(
                        out=f1t[:, fc, :], in_=psf[:],
                        func=mybir.ActivationFunctionType.Relu,
                        bias=b1_fm[:, fc:fc + 1], scale=1.0, alpha=0.0)
                if fb + 1 < NQB:
                    ht_next = _transpose_h(fb + 1)
                urows = []
                for qc in range(4):
                    rc = fb * 4 + qc
                    pso = ps_g.tile([P, D], f32, tag="ps_out2")
                    for fc in range(FC):
                        nc.tensor.matmul(
                            pso[:], lhsT=f1t[:, fc, qc * P:(qc + 1) * P],
                            rhs=w2_sb[:, fc, :],
                            start=(fc == 0), stop=(fc == FC - 1))
                    t = h_full[:, rc, :]
                    if b2_bc is not None:
                        nc.vector.tensor_add(out=pso[:], in0=pso[:],
                                             in1=b2_bc[:])
                    nc.vector.tensor_add(out=t, in0=t, in1=pso[:])
                    urows.append(t)
                ln_batch(ep, urows, g2_bc, be2_bc)
                for qc in range(4):
                    rc = fb * 4 + qc
                    nc.sync.dma_start(out[rc * P:(rc + 1) * P, :],
                                      h_full[:, rc, :])

    nc.finalize()
    return nc


_CACHE = {}
_LOCK = threading.Lock()


def _get_program(spec):
    with _LOCK:
        if spec not in _CACHE:
            _CACHE[spec] = build_program(spec)
        return _CACHE[spec]


def _spec_flags(inputs):
    flags = set()
    if not np.any(inputs["bq"]):
        flags.add("bq0")
    if not np.any(inputs["bv"]):
        flags.add("bv0")
    if not np.any(inputs["b2"]):
        flags.add("b20")
    if (np.all(inputs["gamma1"] == 1.0) and not np.any(inputs["beta1"])):
        flags.add("ln1_triv")
    if (np.all(inputs["gamma2"] == 1.0) and not np.any(inputs["beta2"])):
        flags.add("ln2_triv")
    return frozenset(flags)


def make_in_maps(inputs):
    bf = ml_dtypes.bfloat16
    spec = _spec_flags(inputs)
    # device projection applies G_dev^T to q^T, so upload Wq@Wk^T to get
    # scoresT = k (Wk Wq^T) q^T = K Q^T
    G = (inputs["Wq"].astype(np.float32)
         @ inputs["Wk"].astype(np.float32).T).astype(bf)
    weights = {
        "G": np.ascontiguousarray(G),
        "Wv": np.ascontiguousarray(inputs["Wv"].astype(bf)),
        "W1": np.ascontiguousarray(inputs["W1"].astype(bf)),
        "W2": np.ascontiguousarray(inputs["W2"].astype(bf)),
        "b1": np.ascontiguousarray(inputs["b1"].astype(np.float32)),
    }
    if "bv0" not in spec:
        weights["bv"] = np.ascontiguousarray(inputs["bv"].astype(np.float32))
    if "b20" not in spec:
        weights["b2"] = np.ascontiguousarray(inputs["b2"].astype(np.float32))
    if "ln1_triv" not in spec:
        weights["gamma1"] = np.ascontiguousarray(
            inputs["gamma1"].astype(np.float32))
        weights["beta1"] = np.ascontiguousarray(
            inputs["beta1"].astype(np.float32))
    if "ln2_triv" not in spec:
        weights["gamma2"] = np.ascontiguousarray(
            inputs["gamma2"].astype(np.float32))
        weights["beta2"] = np.ascontiguousarray(
            inputs["beta2"].astype(np.float32))
    wkbq = (None if "bq0" in spec else
            inputs["Wk"].astype(np.float32)
            @ inputs["bq"].astype(np.float32))
    in_maps = []
    for c in range(N_CORES):
        b, h = c // 2, c % 2
        sl = slice(h * M, (h + 1) * M)
        kb = inputs["k"][b].astype(np.float32)
        m = {
            "qT": np.ascontiguousarray(inputs["q"][b, sl].T.astype(bf)),
            "kT": np.ascontiguousarray(kb.T.astype(bf)),
            "v": np.ascontiguousarray(inputs["v"][b].astype(bf)),
            "x": np.ascontiguousarray(inputs["x"][b, sl].astype(np.float32)),
            **weights,
        }
        if wkbq is not None:
            m["abias"] = np.ascontiguousarray(
                ((kb @ wkbq) * SCALE - CSHIFT).astype(np.float32))
        in_maps.append(m)
    return in_maps


def kernel(**inputs):
    spec = _spec_flags(inputs)
    nc = _get_program(spec)
    in_maps = make_in_maps(inputs)
    res = run_bass_kernel_spmd(nc, in_maps, list(range(N_CORES)))
    out = np.empty((B, S, D), np.float32)
    for c in range(N_CORES):
        b, h = c // 2, c % 2
        out[b, h * M:(h + 1) * M] = res.results[c]["out"]
    return out
